# revision 9
# baseline (speedup 1.0000x reference)
"""DeepSeek-V3 token-choice top-k router on 8 Trainium2 NeuronCores.

Strategy (per core, data-parallel over tokens; 1024 tokens/core):
  - x shard [1024, 7168] fp32 streamed from HBM in 8 token-tiles of [128, 7168].
  - Gate weight split on host into fp32r hi/lo pair (exact: hi + lo == w in
    fp32) and packed to [128, 56*256] d-major chunks, replicated per core.
  - PE: per 128-token tile, transpose x chunks ([128t,128d] -> [128d,128t]);
    ACT casts the PSUM transpose to fp32r (hi, round-to-nearest); DVE computes
    lo = x_T - hi (exact Sterbenz subtract, cast fp32r keeps all useful bits).
    Then 3 accumulating fp32r matmuls per chunk (hi@w_hi + hi@w_lo + lo@w_hi;
    the dropped lo@w_lo term is ~2^-26 relative) -> exact-fp32-grade logits
    [128 tokens, 256 experts] in PSUM at 1 cycle/row instead of fp32's 4.
  - ACT: sigmoid(logits) PSUM->SBUF.
  - DVE: hardware top-8 (`max`/`max_index`) for group top-2 sums, top-4 group
    threshold, masked top-8; normalization.
  - GPSIMD: bias add, group masking, and the one-hot weight gathers
    (scalar_tensor_tensor with accumulate), keeping DVE under the PE span.
"""

import numpy as np

N = 8192
D = 7168
E = 256
G = 8
EPG = E // G  # 32
TOPK_GROUP = 4
TOP_K = 8
SCALING = 2.5
N_CORES = 8
NPC = N // N_CORES  # 1024 tokens per core
P = 128
KC = D // P  # 56 contraction chunks
TT = NPC // P  # 8 token tiles per core
KB = 4  # k-chunks per transpose batch (one PSUM bank)
NB = KC // KB  # 14 batches

_CACHE = {}


def build_program(mode="f32r_3pass"):
    import concourse.bacc as bacc
    import concourse.mybir as mybir
    from concourse import tile, masks

    nc = bacc.Bacc(
        "TRN2",
        target_bir_lowering=False,
        debug=False,
        enable_asserts=True,
        num_devices=N_CORES,
    )
    f32 = mybir.dt.float32
    f32r = mybir.dt.float32r
    i32 = mybir.dt.int32
    u32 = mybir.dt.uint32
    AF = mybir.ActivationFunctionType
    OP = mybir.AluOpType
    AX = mybir.AxisListType

    three_pass = mode == "f32r_3pass"
    gdt = f32r if three_pass else f32

    x_d = nc.dram_tensor("x", [NPC, D], f32, kind="ExternalInput").ap()
    if three_pass:
        gwh_d = nc.dram_tensor("gw2", [P, KC * 2 * E], f32r, kind="ExternalInput").ap()
    else:
        gwh_d = nc.dram_tensor("gwh", [P, KC * E], gdt, kind="ExternalInput").ap()
    bias_d = nc.dram_tensor("bias", [1, E], f32, kind="ExternalInput").ap()
    idx_d = nc.dram_tensor("idx", [NPC, TOP_K], i32, kind="ExternalOutput").ap()
    w_d = nc.dram_tensor("w", [NPC, TOP_K], f32, kind="ExternalOutput").ap()

    with tile.TileContext(nc) as tc:
        with (
            tc.tile_pool(name="const", bufs=1) as const_pool,
            tc.tile_pool(name="gw", bufs=1) as gw_pool,
            tc.tile_pool(name="x", bufs=2) as x_pool,
            tc.tile_pool(name="xt", bufs=4) as xt_pool,
            tc.tile_pool(name="ptr", bufs=3, space="PSUM") as ptr_pool,
            tc.tile_pool(name="plog", bufs=2, space="PSUM") as plog_pool,
            tc.tile_pool(name="work", bufs=2) as work_pool,
            tc.tile_pool(name="outs", bufs=2) as out_pool,
        ):
            # ---- tiny bias DMA first, then first x tile, then gw quarters ----
            bias_sb = const_pool.tile([1, E], f32, name="biassb")
            nc.sync.dma_start(bias_sb[:], bias_d[:])
            x_tiles = {}
            x_tiles[0] = x_pool.tile([P, D], f32, tag="xtile", name="xtile0")
            nc.sync.dma_start(x_tiles[0][:], x_d[0:P, :])

            # ---- gate weight DMA in quarters; x1 issued between q1 and q2
            if three_pass:
                gwh_sb = gw_pool.tile([P, KC * 2 * E], f32r, name="gw2sb")
                q = KC * 2 * E // 4
                nc.sync.dma_start(gwh_sb[:, 0 * q : 1 * q], gwh_d[:, 0 * q : 1 * q])
                nc.sync.dma_start(gwh_sb[:, 1 * q : 2 * q], gwh_d[:, 1 * q : 2 * q])
                x_tiles[1] = x_pool.tile([P, D], f32, tag="xtile", name="xtile1")
                nc.sync.dma_start(x_tiles[1][:], x_d[P : 2 * P, :])
                nc.sync.dma_start(gwh_sb[:, 2 * q : 3 * q], gwh_d[:, 2 * q : 3 * q])
                x_tiles[2] = x_pool.tile([P, D], f32, tag="xtile", name="xtile2")
                nc.sync.dma_start(x_tiles[2][:], x_d[2 * P : 3 * P, :])
                nc.sync.dma_start(gwh_sb[:, 3 * q : 4 * q], gwh_d[:, 3 * q : 4 * q])
                gwh_v = gwh_sb[:].rearrange("p (k e) -> p k e", k=KC)
            else:
                gwh_sb = gw_pool.tile([P, KC * E], gdt, name="gwhsb")
                nc.sync.dma_start(gwh_sb[:], gwh_d[:])
                gwh_v = gwh_sb[:].rearrange("p (k e) -> p k e", k=KC)

            # ---- constants ----
            ident = const_pool.tile([P, P], f32)
            masks.make_identity(nc, ident[:])
            iota_i = const_pool.tile([P, E], i32)
            nc.gpsimd.iota(iota_i[:], pattern=[[1, E]], base=0, channel_multiplier=0)
            iota_f = const_pool.tile([P, E], f32)
            nc.vector.tensor_copy(iota_f[:], iota_i[:])
            bias_rep = const_pool.tile([P, E], f32)
            nc.gpsimd.partition_broadcast(bias_rep[:], bias_sb[0:1, :])

            for t in range(TT):
                if t not in x_tiles:
                    x_tiles[t] = x_pool.tile([P, D], f32, tag="xtile", name=f"xtile{t}")
                    nc.sync.dma_start(x_tiles[t][:], x_d[t * P : (t + 1) * P, :])
                if t + 1 < TT and (t + 1) not in x_tiles:
                    x_tiles[t + 1] = x_pool.tile([P, D], f32, tag="xtile", name=f"xtile{t+1}")
                    nc.sync.dma_start(
                        x_tiles[t + 1][:], x_d[(t + 1) * P : (t + 2) * P, :]
                    )
                x_tile = x_tiles[t]

                if three_pass:
                    plog = plog_pool.tile([P, 2 * E], f32, tag="plog")
                else:
                    plog = plog_pool.tile([P, E], f32, tag="plog")
                for b in range(NB):
                    ptr = ptr_pool.tile([P, KB * P], f32, tag="ptr")
                    for j in range(KB):
                        k = b * KB + j
                        nc.tensor.matmul(
                            ptr[:, j * P : (j + 1) * P],
                            x_tile[:, k * P : (k + 1) * P],
                            ident[:],
                            is_transpose=True,
                        )
                    if three_pass:
                        hi_sb = xt_pool.tile([P, KB * P], f32r, tag="hi")
                        nc.scalar.copy(hi_sb[:], ptr[:])
                        lo_sb = xt_pool.tile([P, KB * P], f32r, tag="lo")
                        nc.vector.scalar_tensor_tensor(
                            lo_sb[:], ptr[:], 0.0, hi_sb[:].bitcast(f32),
                            op0=OP.add, op1=OP.subtract,
                        )
                        for j in range(KB):
                            k = b * KB + j
                            sl = slice(j * P, (j + 1) * P)
                            nc.tensor.matmul(
                                plog[:], hi_sb[:, sl], gwh_v[:, k, :],
                                start=(k == 0), stop=False,
                            )
                            nc.tensor.matmul(
                                plog[:, 0:E], lo_sb[:, sl], gwh_v[:, k, 0:E],
                                start=False, stop=(k == KC - 1),
                            )
                    else:
                        xt_sb = xt_pool.tile([P, KB * P], f32, tag="hi")
                        nc.scalar.copy(xt_sb[:], ptr[:])
                        for j in range(KB):
                            k = b * KB + j
                            nc.tensor.matmul(
                                plog[:],
                                xt_sb[:, j * P : (j + 1) * P],
                                gwh_v[:, k, :],
                                start=(k == 0), stop=(k == KC - 1),
                            )

                # ---- routing for this token tile ----
                scores = work_pool.tile([P, E], f32, tag="scores")
                if three_pass:
                    half2 = work_pool.tile([P, E], f32, tag="half2")
                    nc.scalar.copy(half2[:], plog[:, E : 2 * E])
                    lsum = work_pool.tile([P, E], f32, tag="lsum")
                    nc.vector.tensor_tensor(
                        lsum[:], plog[:, 0:E], half2[:], op=OP.add
                    )
                    nc.scalar.activation(scores[:], lsum[:], AF.Sigmoid)
                else:
                    nc.scalar.activation(scores[:], plog[:], AF.Sigmoid)

                sfc = work_pool.tile([P, E], f32, tag="sfc")
                nc.gpsimd.tensor_tensor(sfc[:], scores[:], bias_rep[:], op=OP.add)

                # per-group top-8 (need top-2 of each group of 32)
                gtops = work_pool.tile([P, G * 8], f32, tag="gtops")
                for g in range(G):
                    nc.vector.max(
                        gtops[:, g * 8 : (g + 1) * 8],
                        sfc[:, g * EPG : (g + 1) * EPG],
                    )
                gv = gtops[:].rearrange("p (g k) -> p g k", g=G)
                gs = work_pool.tile([P, G], f32, tag="gs")
                nc.vector.tensor_tensor(gs[:], gv[:, :, 0], gv[:, :, 1], op=OP.add)

                # top-4 groups -> mask
                gtop8 = work_pool.tile([P, 8], f32, tag="gtop8")
                nc.vector.max(gtop8[:], gs[:])
                gmask = work_pool.tile([P, G], f32, tag="gmask")
                nc.vector.tensor_scalar(
                    gmask[:], gs[:], gtop8[:, TOPK_GROUP - 1 : TOPK_GROUP], None,
                    op0=OP.is_ge,
                )

                # masked scores
                tmp = work_pool.tile([P, E], f32, tag="tmp")
                for g in range(G):
                    nc.vector.tensor_scalar(
                        tmp[:, g * EPG : (g + 1) * EPG],
                        sfc[:, g * EPG : (g + 1) * EPG],
                        gmask[:, g : g + 1],
                        None,
                        op0=OP.mult,
                    )

                # top-8 values + indices
                vals = work_pool.tile([P, TOP_K], f32, tag="vals")
                nc.vector.max(vals[:], tmp[:])
                idxu = work_pool.tile([P, TOP_K], u32, tag="idxu")
                nc.vector.max_index(idxu[:], vals[:], tmp[:])
                idxf = work_pool.tile([P, TOP_K], f32, tag="idxf")
                nc.vector.tensor_copy(idxf[:], idxu[:])

                # gather raw sigmoid scores at the selected indices (GPSIMD)
                w8 = out_pool.tile([P, TOP_K], f32, tag="w8")
                scratch = work_pool.tile([P, E], f32, tag="scratch")
                for j in range(TOP_K):
                    nc.vector.scalar_tensor_tensor(
                        scratch[:],
                        iota_f[:],
                        idxf[:, j : j + 1],
                        scores[:],
                        op0=OP.is_equal,
                        op1=OP.mult,
                        accum_out=w8[:, j : j + 1],
                    )

                # normalize + scale
                wsum = work_pool.tile([P, 1], f32, tag="wsum")
                nc.vector.reduce_sum(wsum[:], w8[:], axis=AX.X)
                wse = work_pool.tile([P, 1], f32, tag="wse")
                nc.vector.tensor_scalar(wse[:], wsum[:], 1e-20, None, op0=OP.add)
                wrec = work_pool.tile([P, 1], f32, tag="wrec")
                nc.vector.reciprocal(wrec[:], wse[:])
                w_out = out_pool.tile([P, TOP_K], f32, tag="wout")
                nc.vector.tensor_scalar(
                    w_out[:], w8[:], wrec[:, 0:1], float(SCALING),
                    op0=OP.mult, op1=OP.mult,
                )
                idx_out = out_pool.tile([P, TOP_K], i32, tag="idxout")
                nc.vector.tensor_copy(idx_out[:], idxu[:])

                nc.sync.dma_start(idx_d[t * P : (t + 1) * P, :], idx_out[:])
                nc.sync.dma_start(w_d[t * P : (t + 1) * P, :], w_out[:])

    nc.compile()
    return nc


def _get_nc(**kw):
    key = tuple(sorted(kw.items()))
    if key not in _CACHE:
        _CACHE[key] = build_program(**kw)
    return _CACHE[key]


def _fp32r_round(a):
    # round-to-nearest fp32 -> fp32r (12-bit mantissa), bit-exact with HW cast
    bits = np.ascontiguousarray(a).view(np.uint32)
    keep = np.uint32(0xFFFFF000)
    rounded = (bits + np.uint32(0x800)) & keep  # round-half-up into kept bits
    # correct round-to-nearest-even on the halfway case
    half = (bits & np.uint32(0xFFF)) == np.uint32(0x800)
    even = ((bits >> np.uint32(12)) & np.uint32(1)) == 0
    rounded = np.where(half & even, bits & keep, rounded)
    return rounded.view(np.float32).reshape(a.shape)


def _pack(a2d):
    # [D, E] -> [P, KC*E]: partition p holds rows k*128+p
    return np.ascontiguousarray(
        a2d.reshape(KC, P, E).transpose(1, 0, 2)
    ).reshape(P, KC * E)


def _run(x, gate_w, bias, trace=False, **build_kw):
    from concourse.bass_utils import run_bass_kernel_spmd

    x = np.ascontiguousarray(np.asarray(x, dtype=np.float32))
    gate_w = np.ascontiguousarray(np.asarray(gate_w, dtype=np.float32))
    bias = np.ascontiguousarray(np.asarray(bias, dtype=np.float32))
    nc = _get_nc(**build_kw)
    mode = build_kw.get("mode", "f32r_3pass")
    gwt = np.ascontiguousarray(gate_w.T)  # [D, E]
    bias2d = bias.reshape(1, E)
    if mode == "f32r_3pass":
        gw_hi = _fp32r_round(gwt)
        gw_lo = _fp32r_round(gwt - gw_hi)
        ph = _pack(gw_hi).reshape(P, KC, E)
        pl = _pack(gw_lo).reshape(P, KC, E)
        gw2 = np.concatenate([ph, pl], axis=2).reshape(P, KC * 2 * E)
        maps = {"gw2": np.ascontiguousarray(gw2), "bias": bias2d}
    else:
        maps = {"gwh": _pack(gwt), "bias": bias2d}
    in_maps = [
        {"x": x[c * NPC : (c + 1) * NPC], **maps} for c in range(N_CORES)
    ]
    res = run_bass_kernel_spmd(nc, in_maps, core_ids=list(range(N_CORES)), trace=trace)
    idx = np.concatenate([res.results[c]["idx"] for c in range(N_CORES)], axis=0)
    w = np.concatenate([res.results[c]["w"] for c in range(N_CORES)], axis=0)
    return (idx.astype(np.int32), w.astype(np.float32)), res


def kernel(x, gate_w, bias):
    (idx, w), _ = _run(x, gate_w, bias)
    return idx, w
